# revision 35
# baseline (speedup 1.0000x reference)
"""Causal multi-head attention block (B=8, T=2048, C=768, H=8) on 8 trn2 cores.

Sharding: data-parallel over batch — one batch element per NeuronCore, weights
replicated, no collectives.

Host prep (numpy): x, w_attn, w_proj pre-transposed; the 1/sqrt(hs) logit
scale folded into the Q weights/bias; the K bias dropped (it only shifts each
softmax row by a constant, which cancels); the V bias folded through w_proj
into the output bias; x and all matmul weights cast to bf16.  The device
kernel therefore has no transpose phase and minimal bias work.

Per-core algorithm (all PE accumulation in f32 PSUM):
  Stream x^T / w^T slices straight into SBUF (QK weights streamed per
  head-pair).  V = x @ w_v^T for all 8 heads in natural [t, d] bf16 layout
  with a ones column per head (softmax denominator); eviction on ACT.
  QK projection per head-pair: 3 full-width M=128 matmul slabs per 512
  t-columns; slab rows scattered to head-aligned q^T/k^T bf16 tiles in
  quadrant-legal partition segments on DVE.
  Per head, per 1024-row half: causal attention in S^T layout: S^T[j, i]
  matmul -> P = exp(S^T) on ACT (bf16) -> diagonal-block mask multiply on
  gpsimd -> O^T[d, i] (+ denominator row l) accumulated in PSUM over j-tiles
  via lhsT=[V|1].  Epilogue: copy O_ps to SBUF on DVE, broadcast l across
  partitions with a selector matmul, reciprocal_approx_fast on DVE, normalize
  on DVE into a staging tile, then DMA-repack into a 128-row-aligned
  resident O^T stripe tile (no DRAM spill; engines cannot cross partition
  quadrants but DMA can).
  Output projection from SBUF with K=128 stripe contraction, two t-tiles per
  out DMA: out = O^T.T @ w_proj^T + b_out, written [t, c].
"""

import math
import os
import sys
from contextlib import ExitStack

for _p in ("/opt/trn_rl_repo", "/root/.axon_site/_ro/trn_rl_repo"):
    if os.path.isdir(_p) and _p not in sys.path:
        sys.path.append(_p)

import numpy as np
import ml_dtypes

import concourse.bass as bass  # noqa: F401  (import keeps bass registered)
from concourse import bacc
import concourse.mybir as mybir
import concourse.tile as tile
from concourse.bass_utils import run_bass_kernel_spmd

F32 = mybir.dt.float32
F32R = mybir.dt.float32r
BF16 = mybir.dt.bfloat16
EXP = mybir.ActivationFunctionType.Exp
ADD = mybir.AluOpType.add
MULT = mybir.AluOpType.mult

B, T, C, H, HS = 8, 2048, 768, 8, 96
KT = C // 128        # 6 contraction tiles of 128
TT = T // 128        # 16 t-tiles of 128
NCORES = 8
BF16_NP = ml_dtypes.bfloat16

# head h occupies rows 96h..96h+95 of the packed [768, T] O^T; as 6 stripes of
# 128 partitions each head maps to 1-2 (stripe, row0, d0, length) segments
_OSEGS = []
for _h in range(H):
    g0 = _h * HS
    s0, r0 = g0 // 128, g0 % 128
    if r0 + HS <= 128:
        _OSEGS.append([(s0, r0, 0, HS)])
    else:
        n0 = 128 - r0
        _OSEGS.append([(s0, r0, 0, n0), (s0 + 1, 0, n0, HS - n0)])


def _chunks(lo, hi, align=512):
    """Split [lo, hi) at multiples of `align`."""
    out = []
    a = lo
    while a < hi:
        b = min(hi, (a // align + 1) * align)
        out.append((a, b))
        a = b
    return out


def build_nc():
    nc = bacc.Bacc()
    xT_d = nc.dram_tensor("xT", [C, T], BF16, kind="ExternalInput")
    wqk_d = nc.dram_tensor("wqk", [C, H // 2, 4 * HS], BF16,
                           kind="ExternalInput")
    wv_d = nc.dram_tensor("wv", [C, C], BF16, kind="ExternalInput")
    wpT_d = nc.dram_tensor("wpT", [C, C], BF16, kind="ExternalInput")
    mk_d = nc.dram_tensor("mk", [128, 128], BF16, kind="ExternalInput")
    bsel_d = nc.dram_tensor("bsel", [HS + 1, HS], F32R, kind="ExternalInput")
    bqk_d = nc.dram_tensor("bqk", [HS, 16], F32, kind="ExternalInput")
    bo_d = nc.dram_tensor("bo", [128, C], F32, kind="ExternalInput")
    out = nc.dram_tensor("out", [T, C], F32, kind="ExternalOutput")

    xT_r = xT_d.rearrange("(k p) t -> p k t", p=128)
    wqk_r = wqk_d.rearrange("(k p) h c -> p k h c", p=128)
    wv_r = wv_d.rearrange("(k p) c -> p k c", p=128)
    wpT_r = wpT_d.rearrange("(k p) c -> p k c", p=128)
    out_r2 = out.rearrange("(g a p) c -> p g a c", a=2, p=128)

    with tile.TileContext(nc) as tc, ExitStack() as ctx:
        consts = ctx.enter_context(tc.tile_pool(name="consts", bufs=1))
        mk_sb = consts.tile([128, 128], BF16, tag="mk")
        bs_sb = consts.tile([HS + 1, HS], F32R, tag="bs")
        bqk_sb = consts.tile([HS, 16], F32, tag="bqk")
        bo_sb = consts.tile([128, C], F32, tag="bo")
        wpT_sb = consts.tile([128, KT, C], BF16, tag="wpT")
        one_bf = consts.tile([128, 1], BF16, tag="one")
        nc.vector.memset(one_bf[:], 1.0)

        xTp = ctx.enter_context(tc.tile_pool(name="xT", bufs=1))
        xT = xTp.tile([128, KT, T], BF16, tag="xT")
        Vp = ctx.enter_context(tc.tile_pool(name="V", bufs=1))
        V = Vp.tile([128, TT, H, HS + 1], BF16, tag="V")
        ostp = ctx.enter_context(tc.tile_pool(name="ost", bufs=1))
        Ost = ostp.tile([128, KT, T], BF16, tag="ost")

        with tc.tile_pool(name="wv", bufs=2) as wvp, \
             tc.tile_pool(name="wqk", bufs=2) as wqkp, \
             tc.tile_pool(name="qk", bufs=8) as qkp, \
             tc.tile_pool(name="pt", bufs=3) as ptp, \
             tc.tile_pool(name="lt", bufs=2) as ltp, \
             tc.tile_pool(name="rc", bufs=2) as rcp, \
             tc.tile_pool(name="stg", bufs=2) as stgp, \
             tc.tile_pool(name="ob", bufs=3) as obp, \
             tc.tile_pool(name="pj", bufs=2, space="PSUM") as pjp, \
             tc.tile_pool(name="sp", bufs=2, space="PSUM") as spp, \
             tc.tile_pool(name="op", bufs=2, space="PSUM") as opp:

            # ---- input DMAs, ordered for the pipeline ----
            # the first V-projection needs bv + wv0 + the first x^T columns;
            # wv1 right behind so the second head-group is never starved;
            # everything else ordered by first use.
            wv_sb = [wvp.tile([128, KT, 4 * HS], BF16, tag="wv",
                              name=f"wv{g}") for g in range(2)]
            nc.sync.dma_start(wv_sb[0][:], wv_r[:, :, 0:4 * HS])
            nc.sync.dma_start(xT[:, :, 0:256], xT_r[:, :, 0:256])
            nc.sync.dma_start(xT[:, :, 256:512], xT_r[:, :, 256:512])
            nc.sync.dma_start(wv_sb[1][:], wv_r[:, :, 4 * HS:8 * HS])
            wqk_sb = {0: wqkp.tile([128, KT, 4 * HS], BF16, tag="wqk",
                                   name="wqkp0")}
            nc.sync.dma_start(wqk_sb[0][:], wqk_r[:, :, 0, :])
            nc.sync.dma_start(bqk_sb[:], bqk_d[:, :])  # q bias only
            nc.sync.dma_start(xT[:, :, 512:1024], xT_r[:, :, 512:1024])
            nc.sync.dma_start(mk_sb[:], mk_d[:, :])
            nc.sync.dma_start(xT[:, :, 1024:1536], xT_r[:, :, 1024:1536])
            nc.sync.dma_start(xT[:, :, 1536:2048], xT_r[:, :, 1536:2048])
            nc.sync.dma_start(bs_sb[:], bsel_d[:, :])
            nc.sync.dma_start(bo_sb[:], bo_d[:, :])
            nc.sync.dma_start(wpT_sb[:], wpT_r[:, :, :])

            # ones columns for the softmax denominator
            nc.vector.tensor_copy(V[:, :, :, HS:HS + 1],
                                  one_bf[:].to_broadcast([128, TT, H, 1]))

            # head-pair QK projection: heads (2p, 2p+1) have 384 contiguous
            # weight columns (q|k|q|k); 3 full-width M=128 matmul slabs per
            # 512 t-columns instead of 4 M=96 ones.  The packed slab rows are
            # scattered back to head-aligned q^T/k^T tiles in quadrant-legal
            # partition segments.  The k bias is dropped entirely: it only
            # shifts each softmax row by a constant, which cancels.
            # (slab, src_row, dst_tile, dst_row, rows, is_q)
            QK_SEGS = [
                (0,  0, 0,  0, 96, True),
                (0, 96, 1,  0, 32, False),
                (1,  0, 1, 32, 32, False),
                (1, 32, 1, 64, 32, False),
                (1, 64, 2,  0, 64, True),
                (2,  0, 2, 64, 32, True),
                (2, 32, 3,  0, 32, False),
                (2, 64, 3, 32, 32, False),
                (2, 96, 3, 64, 32, False),
            ]

            def proj_pair(p, tc4):
                """QK projection for heads (2p, 2p+1), t-cols [512*tc4, +512)."""
                dsts = qkh[2 * p] + qkh[2 * p + 1]
                t0 = tc4 * 512
                for sl in range(3):
                    pjs = pjp.tile([128, 512], F32, tag="pj", name="pjs")
                    for kc in range(KT):
                        nc.tensor.matmul(
                            pjs[:, 0:512],
                            wqk_sb[p][:, kc, 128 * sl:128 * (sl + 1)],
                            xT[:, kc, t0:t0 + 512],
                            start=(kc == 0), stop=(kc == KT - 1))
                    for (s, s0, di, d0, ln, is_q) in QK_SEGS:
                        if s != sl:
                            continue
                        dst = dsts[di][d0:d0 + ln, t0:t0 + 512]
                        if is_q:
                            hq = 2 * p + di // 2
                            nc.vector.tensor_tensor(
                                dst, pjs[s0:s0 + ln, 0:512],
                                bqk_sb[d0:d0 + ln, hq:hq + 1]
                                    .to_broadcast([ln, 512]),
                                ADD)
                        else:
                            nc.vector.tensor_copy(dst, pjs[s0:s0 + ln, 0:512])

            # qk tiles: [qT, kT] per head; current pair + next pair live
            qkh = {}

            def alloc_pair(p):
                for h in (2 * p, 2 * p + 1):
                    qkh[h] = [qkp.tile([HS, T], BF16, tag="qk",
                                       name=f"qk{h}_{i}") for i in range(2)]

            wqk_sb[1] = wqkp.tile([128, KT, 4 * HS], BF16, tag="wqk",
                                  name="wqkp1")
            nc.sync.dma_start(wqk_sb[1][:], wqk_r[:, :, 1, :])

            # ---- V projection (all heads) + pair-0 QK proj, pipelined with
            # the x^T chunk DMAs ----
            alloc_pair(0)
            for ch in range(4):
                for g in range(2):
                    for tt in range(4 * ch, 4 * ch + 4):
                        vps = pjp.tile([128, 512], F32, tag="pj", name="vps")
                        for kc in range(KT):
                            nc.tensor.matmul(
                                vps[:, 0:4 * HS],
                                xT[:, kc, tt * 128:(tt + 1) * 128],
                                wv_sb[g][:, kc, :],
                                start=(kc == 0), stop=(kc == KT - 1))
                        nc.scalar.activation(
                            V[:, tt, 4 * g:4 * g + 4, 0:HS],
                            vps[:, 0:4 * HS].rearrange("p (h d) -> p h d",
                                                       d=HS),
                            mybir.ActivationFunctionType.Copy)
                proj_pair(0, ch)

            def attn_half(h, ihalf, after_jt0=None):
                """S^T/P/O^T accumulation for rows [1024*ihalf, +1024)."""
                qT, kT = qkh[h]
                ibase = 1024 * ihalf
                iend = ibase + 1024
                njt = 8 * (ihalf + 1)
                O_ps = opp.tile([128, 1024], F32, tag="O", name="O_ps")
                # j-tiles in REVERSE order: the tiny diagonal-tail tiles (a
                # fully serial S->exp->mask->O chain each) run early, hidden
                # under the pipeline; each half ends with the big well-
                # overlapped tiles.  Correct under PSUM accumulate-bit
                # semantics: the first-processed (highest) j-tile per column
                # chunk carries start=True (clearing the bank), later tiles
                # overwrite columns not yet written and accumulate the rest.
                for jt in reversed(range(njt)):
                    j0 = 128 * jt
                    i0 = max(j0, ibase)
                    ilen = iend - i0
                    P = ptp.tile([128, 1024], BF16, tag="P", name="P")
                    for (ra, rb) in _chunks(0, ilen):
                        S = spp.tile([128, 512], F32, tag="S", name="S")
                        nc.tensor.matmul(S[:, 0:rb - ra],
                                         kT[0:HS, j0:j0 + 128],
                                         qT[0:HS, i0 + ra:i0 + rb],
                                         start=True, stop=True)
                        nc.scalar.activation(P[:, ra:rb], S[:, 0:rb - ra],
                                             EXP)
                    if j0 >= ibase:
                        nc.gpsimd.tensor_tensor(P[:, 0:128], P[:, 0:128],
                                                mk_sb[:], MULT)
                    for (a, b) in _chunks(i0, iend):
                        ci = a // 512
                        first_jt = min(4 * ci + 3, njt - 1)
                        nc.tensor.matmul(
                            O_ps[0:HS + 1, a - ibase:b - ibase],
                            V[:, jt, h, :],
                            P[:, a - i0:b - i0],
                            start=(jt == first_jt), stop=(jt == 0),
                            skip_group_check=True)
                    if jt == njt - 1 and after_jt0 is not None:
                        after_jt0()
                return O_ps

            def epilogue(h, ihalf, O_ps):
                """Normalize O^T by the denominator row (row HS of O_ps) and
                write the bf16 result into the 128-row-aligned stripes."""
                ibase = 1024 * ihalf
                iend = ibase + 1024
                lt = ltp.tile([HS + 1, 1024], F32R, tag="lt", name="lt")
                nc.vector.tensor_copy(lt[:], O_ps[0:HS + 1, :])
                Lp = opp.tile([128, 1024], F32, tag="O", name="Lp")
                R = rcp.tile([HS, 1024], F32, tag="R", name="R")
                for (ra, rb) in ((0, 512), (512, 1024)):
                    nc.tensor.matmul(Lp[0:HS, ra:rb], bs_sb[:, :],
                                     lt[:, ra:rb], start=True, stop=True)
                    nc.vector.reciprocal_approx_fast(R[:, ra:rb],
                                                     Lp[0:HS, ra:rb])
                stg = stgp.tile([HS, 1024], BF16, tag="stg", name="stg")
                # normalize on DVE: keeps the gpsimd queue to fast mask ops
                # so diagonal-block O matmuls never wait behind a 2.4us MULT
                nc.vector.tensor_tensor(stg[:], lt[0:HS, :], R[:], MULT)
                # repack into the 128-row-aligned stripes via DMA (engines
                # cannot cross partition quadrants; DMA can)
                for (s, r0, d0, ln) in _OSEGS[h]:
                    nc.sync.dma_start(Ost[r0:r0 + ln, s, ibase:iend],
                                      stg[d0:d0 + ln, :])

            # ---- per-head attention ----
            for h in range(H):
                if h % 2 == 0 and h // 2 + 2 < H // 2:
                    pnx = h // 2 + 2
                    wqk_sb[pnx] = wqkp.tile([128, KT, 4 * HS], BF16,
                                            tag="wqk", name=f"wqkp{pnx}")
                    nc.sync.dma_start(wqk_sb[pnx][:], wqk_r[:, :, pnx, :])

                O0 = attn_half(h, 0)
                O1 = attn_half(h, 1,
                               after_jt0=lambda: epilogue(h, 0, O0))
                if h % 2 == 0 and h + 2 < H:
                    alloc_pair(h // 2 + 1)
                    for tc4 in range(4):
                        proj_pair(h // 2 + 1, tc4)
                epilogue(h, 1, O1)

            # ---- output projection, from SBUF, K=128 stripes ----
            # two t-tiles per staging tile -> 8 fat out DMAs (fewer
            # descriptors + completion semaphores on the tail)
            for tg in range(TT // 2):
                o_sb = obp.tile([128, 2, C], F32, tag="o", name="o_sb")
                for ta in range(2):
                    tt = 2 * tg + ta
                    cps = opp.tile([128, 1024], F32, tag="O", name="cps")
                    for (a, b) in ((0, 512), (512, C)):
                        for kc in range(KT):
                            nc.tensor.matmul(
                                cps[:, a:b],
                                Ost[:, kc, tt * 128:(tt + 1) * 128],
                                wpT_sb[:, kc, a:b],
                                start=(kc == 0), stop=(kc == KT - 1))
                    nc.vector.tensor_tensor(o_sb[:, ta, :], cps[:, 0:C],
                                            bo_sb[:], ADD)
                nc.gpsimd.dma_start(out_r2[:, tg], o_sb[:])

    nc.finalize()
    return nc


_NC_CACHE = {}


def _get_nc():
    if "nc" not in _NC_CACHE:
        _NC_CACHE["nc"] = build_nc()
    return _NC_CACHE["nc"]


def _make_consts(w_attn, b_attn, w_proj, b_proj):
    s = 1.0 / math.sqrt(HS)
    waT = np.ascontiguousarray(w_attn.T)          # [C, 3C]
    wqk = np.empty((C, H, 2 * HS), dtype=np.float32)
    for h in range(H):
        wqk[:, h, 0:HS] = waT[:, h * HS:(h + 1) * HS] * s
        wqk[:, h, HS:2 * HS] = waT[:, C + h * HS:C + (h + 1) * HS]
    wqk = wqk.reshape(C, H // 2, 4 * HS).astype(BF16_NP)  # head-pair packing
    wv = np.ascontiguousarray(waT[:, 2 * C:3 * C]).astype(BF16_NP)
    wpT = np.ascontiguousarray(w_proj.T).astype(BF16_NP)
    bqk = np.empty((HS, 16), dtype=np.float32)
    for m in range(8):
        bqk[:, m] = b_attn[m * HS:(m + 1) * HS] * s
    for m in range(8):
        bqk[:, 8 + m] = b_attn[C + m * HS:C + (m + 1) * HS]
    bo_row = b_proj + b_attn[2 * C:3 * C] @ w_proj.T
    bo = np.ascontiguousarray(
        np.broadcast_to(bo_row, (128, C)).astype(np.float32))
    mk = np.triu(np.ones((128, 128), dtype=np.float32)).astype(BF16_NP)
    bsel = np.zeros((HS + 1, HS), dtype=np.float32)
    bsel[HS, :] = 1.0
    return wqk, wv, wpT, bqk, bo, mk, bsel


def kernel(x, w_attn, b_attn, w_proj, b_proj, _want_results=False, **run_kwargs):
    x = np.asarray(x, dtype=np.float32)
    w_attn = np.asarray(w_attn, dtype=np.float32)
    b_attn = np.asarray(b_attn, dtype=np.float32)
    w_proj = np.asarray(w_proj, dtype=np.float32)
    b_proj = np.asarray(b_proj, dtype=np.float32)

    wqk, wv, wpT, bqk, bo, mk, bsel = _make_consts(
        w_attn, b_attn, w_proj, b_proj)

    nc = _get_nc()
    common = dict(wqk=wqk, wv=wv, wpT=wpT, bqk=bqk, bo=bo, mk=mk,
                  bsel=bsel)
    in_maps = [dict(xT=np.ascontiguousarray(x[c].T).astype(BF16_NP), **common)
               for c in range(NCORES)]
    res = run_bass_kernel_spmd(nc, in_maps, core_ids=list(range(NCORES)),
                               **run_kwargs)
    out = np.stack([res.results[c]["out"] for c in range(NCORES)], axis=0)
    if _want_results:
        return out, res
    return out


if __name__ == "__main__":
    rng = np.random.default_rng(0)
    x = rng.standard_normal((B, T, C), dtype=np.float32)
    w_attn = rng.standard_normal((3 * C, C), dtype=np.float32) / math.sqrt(C)
    b_attn = rng.standard_normal(3 * C).astype(np.float32) * 0.02
    w_proj = rng.standard_normal((C, C), dtype=np.float32) / math.sqrt(C)
    b_proj = rng.standard_normal(C).astype(np.float32) * 0.02
    o = kernel(x, w_attn, b_attn, w_proj, b_proj)
    print("out", o.shape, o.dtype, float(np.abs(o).mean()))
